# revision 1
# baseline (speedup 1.0000x reference)
"""Trainium2 Bass kernel for nn_AttMatch (2-graph attention + SAGEConv GNN).

Self-contained: takes the full unsharded inputs of the reference problem,
shards across 8 NeuronCores internally, runs one SPMD NEFF, and gathers the
full [8192, 8192] sigmoid adjacency output.

Sharding: the concatenated target set (2*4096 rows) is row-sharded across the
8 cores (512 rows of each graph per core).  Key/value and the attention
matrix are sharded along T; the softmax over dim 0 and alpha.T @ v are
completed with a ReduceScatter (global-chunk ordered, so every core receives
exactly its own node chunk).

SAGEConv (mean aggregation, M = row-normalized adjacency operator, built on
host from the edge index) is split algebraically so that only the
attention-output-dependent part sits on the critical path:

    h = relu( M @ X @ (Wl0+Wl1) + X @ (Wr0+Wr1) + bl     <- X-only, overlaps
              - M @ (out @ Wl1) - out_own @ Wr1 )           attention
                ^^^^^^^^^^^^^^^ distributed: each core computes its
                row-block partial W_c = M[:, own_j] @ (out_own @ Wl1) and a
                second ReduceScatter sums and re-shards them.

Node features are re-replicated with one AllGather per graph per layer.
"""

import numpy as np
import ml_dtypes

import concourse.bass as bass
import concourse.bacc as bacc
import concourse.tile as tile
import concourse.mybir as mybir
from concourse.bass_utils import run_bass_kernel_spmd

BF16 = ml_dtypes.bfloat16

N = 4096          # nodes per graph
D = 128           # feature dim (in == out == 128)
NCORES = 8
SH = N // NCORES  # 512 node shard per graph per core
ICW = 1024        # query-chunk width
NIC = N // ICW    # 4 query chunks
NT = 2 * SH // 128  # 8 local target tiles of 128 (512 of each graph)
NJ = N // 128     # 32 source-node tiles
INV_SCALE = 1.0 / np.sqrt(128.0)

F32 = mybir.dt.float32
BF = mybir.dt.bfloat16

ADD = mybir.AluOpType.add
SUB = mybir.AluOpType.subtract
MULT = mybir.AluOpType.mult
MAX = mybir.AluOpType.max

# wm indices (per layer l: base = 7*l)
WK, WQ, WV, WLS, WL1, WRS, WR1N = range(7)
IDENT = 14
# bias indices (per layer l: base = 4*l)
BK, BQ, BV, BL = range(4)

_cache = {}


def _build_nc():
    """Build and compile the SPMD Bass graph (one NeuronCore program)."""
    nc = bacc.Bacc("TRN2", target_bir_lowering=False, debug=False,
                   num_devices=NCORES)

    # ---- external I/O ----
    x1t = nc.dram_tensor("x1t", [D, N], BF, kind="ExternalInput")
    x2t = nc.dram_tensor("x2t", [D, N], BF, kind="ExternalInput")
    xgt_in = [x1t, x2t]
    xown_in = nc.dram_tensor("xown", [2, D, SH], BF, kind="ExternalInput")
    # column shard of M^T (for the own-column aggregation)
    mtc_in = [nc.dram_tensor("mtc1", [NJ, 128, SH], BF, kind="ExternalInput"),
              nc.dram_tensor("mtc2", [NJ, 128, SH], BF, kind="ExternalInput")]
    wm_in = nc.dram_tensor("wm", [15, 128, 128], BF, kind="ExternalInput")
    bs_in = nc.dram_tensor("bs", [8, 128, 1], F32, kind="ExternalInput")
    out_ext = nc.dram_tensor("out", [2, SH, 2 * N], F32, kind="ExternalOutput")

    # ---- internal DRAM for collectives ----
    rg = [list(range(NCORES))]
    ar_in = [[nc.dram_tensor(f"ar_in_{l}_{g}", [2, 129, N // 2], BF)
              for g in range(2)] for l in range(2)]
    ar_out = [[nc.dram_tensor(f"ar_out_{l}_{g}", [2, 129, N // 2], BF,
                              addr_space="Shared")
               for g in range(2)] for l in range(2)]
    hag_in = [[nc.dram_tensor(f"hag_in_{l}_{g}", [D, SH], BF)
               for g in range(2)] for l in range(2)]
    hag_out = [[nc.dram_tensor(f"hag_out_{l}_{g}", [NCORES, D, SH], BF,
                               addr_space="Shared")
                for g in range(2)] for l in range(2)]

    with tile.TileContext(nc) as tc:
        with (
            tc.tile_pool(name="const", bufs=1) as cpool,
            tc.tile_pool(name="xt", bufs=2) as xt_pool,
            tc.tile_pool(name="small", bufs=2) as spool,
            tc.tile_pool(name="kqv", bufs=1) as kqv_pool,
            tc.tile_pool(name="es", bufs=6) as es_pool,
            tc.tile_pool(name="csacc", bufs=2) as cs_pool,
            tc.tile_pool(name="stage", bufs=3) as st_pool,
            tc.tile_pool(name="ybig", bufs=1) as y_pool,
            tc.tile_pool(name="mt", bufs=1) as mt_pool,
            tc.tile_pool(name="z", bufs=5) as z_pool,
            tc.tile_pool(name="ps", bufs=2, space="PSUM") as ps_pool,
            tc.tile_pool(name="ps_p", bufs=3, space="PSUM") as psp_pool,
            tc.tile_pool(name="ps_cs", bufs=1, space="PSUM") as pscs_pool,
        ):
            # ---- load constants ----
            wm = cpool.tile([128, 15 * 128], BF, name="wm_sb")
            nc.scalar.dma_start(
                wm.rearrange("p (i f) -> p i f", i=15),
                wm_in.ap().rearrange("i p f -> p i f"))
            bs = cpool.tile([128, 8], F32, name="bs_sb")
            nc.scalar.dma_start(
                bs.rearrange("p (i f) -> p i f", i=8),
                bs_in.ap().rearrange("i p f -> p i f"))
            ones_m1 = cpool.tile([128, 1], BF, name="ones_m1")
            nc.vector.memset(ones_m1[:], 1.0)
            ones_row = cpool.tile([1, 128], BF, name="ones_row")
            nc.vector.memset(ones_row[:], 1.0)

            def W(l, i):
                base = 7 * l + i if i < 7 else IDENT
                return wm[:, 128 * base:128 * (base + 1)]

            def B(l, i):
                return bs[:, 4 * l + i:4 * l + i + 1]

            ident = wm[:, 128 * IDENT:128 * (IDENT + 1)]

            # ---- load inputs (generation 0) ----
            xgt = []
            for g in range(2):
                t = xt_pool.tile([D, N], BF, name=f"x{g}t_0", tag=f"xt{g}")
                nc.scalar.dma_start(t[:], xgt_in[g][:])
                xgt.append(t)
            xown = []
            for g in range(2):
                t = spool.tile([D, SH], BF, name=f"xown{g}_0", tag=f"xo{g}")
                nc.scalar.dma_start(t[:], xown_in[g])
                xown.append(t)

            hown_final = [None, None]

            for l in range(2):
                # ---- projections ----
                kt = kqv_pool.tile([D, 2 * SH], BF, name=f"kt_{l}", tag="kt",
                                   bufs=2)
                vnat = []
                for g in range(2):
                    ps = ps_pool.tile([128, 512], F32, tag="ps",
                                      name=f"psk_{l}_{g}")
                    nc.tensor.matmul(ps[:], W(l, WK), xown[g][:],
                                     start=True, stop=True)
                    nc.vector.tensor_scalar(kt[:, g * SH:(g + 1) * SH], ps[:],
                                            B(l, BK), None, ADD)
                    # v^T then transpose to natural [t, d] tiles (no bias; bv
                    # is folded in after the softmax division)
                    ps2 = ps_pool.tile([128, 512], F32, tag="ps",
                                       name=f"psv_{l}_{g}")
                    nc.tensor.matmul(ps2[:], W(l, WV), xown[g][:],
                                     start=True, stop=True)
                    vt = st_pool.tile([128, SH], BF, name=f"vt_{l}_{g}",
                                      tag="vt", bufs=2)
                    nc.vector.tensor_copy(vt[:], ps2[:])
                    psv = ps_pool.tile([128, 512], BF, tag="ps",
                                       name=f"psvt_{l}_{g}")
                    for j in range(4):
                        nc.tensor.transpose(psv[:, j * 128:(j + 1) * 128],
                                            vt[:, j * 128:(j + 1) * 128],
                                            ident)
                    vb = st_pool.tile([128, 512], BF, name=f"vn_{l}_{g}",
                                      tag=f"vn{g}", bufs=1)
                    nc.vector.tensor_copy(vb[:], psv[:])
                    vnat += [vb[:, j * 128:(j + 1) * 128] for j in range(4)]
                qt = []
                for g in range(2):
                    q = kqv_pool.tile([D, N], BF, name=f"qt_{l}_{g}",
                                      tag=f"qt{g}", bufs=1)
                    for ic in range(NIC):
                        ps = ps_pool.tile([128, ICW], F32, tag="ps",
                                          name=f"psq_{l}_{g}_{ic}")
                        for h in range(2):
                            nc.tensor.matmul(
                                ps[:, h * 512:(h + 1) * 512], W(l, WQ),
                                xgt[g][:, ic * ICW + h * 512:
                                        ic * ICW + (h + 1) * 512],
                                start=True, stop=True)
                        nc.vector.tensor_scalar(q[:, ic * ICW:(ic + 1) * ICW],
                                                ps[:], B(l, BQ), None, ADD)
                    qt.append(q)

                ls = [None, None]      # local X-only SAGE terms [d, own]
                hown = [None, None]

                def attention(g):
                    for ic in range(NIC):
                        php = [psp_pool.tile([128, 512], F32, tag="ps_p",
                                             name=f"php{h}_{l}_{g}_{ic}")
                               for h in range(2)]
                        csa = cs_pool.tile([128, ICW], BF, tag="cs")
                        for tt in range(NT):
                            ps_s = ps_pool.tile([128, ICW], F32, tag="ps")
                            for h in range(2):
                                nc.tensor.matmul(
                                    ps_s[:, h * 512:(h + 1) * 512],
                                    kt[:, tt * 128:(tt + 1) * 128],
                                    qt[g][:, ic * ICW + h * 512:
                                            ic * ICW + (h + 1) * 512],
                                    start=True, stop=True)
                            es = es_pool.tile([128, ICW], BF, tag="es")
                            nc.scalar.activation(
                                es[:], ps_s[:],
                                mybir.ActivationFunctionType.Exp,
                                scale=INV_SCALE)
                            for h in range(2):
                                nc.tensor.matmul(
                                    php[h][:], vnat[tt],
                                    es[:, h * 512:(h + 1) * 512],
                                    start=(tt == 0), stop=(tt == NT - 1))
                            if tt == 0:
                                nc.vector.tensor_copy(csa[:], es[:])
                            else:
                                nc.vector.tensor_tensor(csa[:], csa[:], es[:],
                                                        ADD)
                        pc = st_pool.tile([128, ICW], BF, tag="pc")
                        cc = st_pool.tile([1, ICW], BF, tag="cc")
                        for h in range(2):
                            nc.vector.tensor_copy(
                                pc[:, h * 512:(h + 1) * 512], php[h][:])
                            ps_c = pscs_pool.tile([1, 512], F32, tag="ps_cs")
                            nc.tensor.matmul(ps_c[:], ones_m1[:],
                                             csa[:, h * 512:(h + 1) * 512],
                                             start=True, stop=True)
                            nc.vector.tensor_copy(
                                cc[:, h * 512:(h + 1) * 512], ps_c[:])
                        hh, icq = divmod(ic, 2)
                        nc.scalar.dma_start(
                            ar_in[l][g][hh, 0:128,
                                        icq * ICW:(icq + 1) * ICW],
                            pc[:])
                        nc.scalar.dma_start(
                            ar_in[l][g][hh, 128:129,
                                        icq * ICW:(icq + 1) * ICW],
                            cc[:])
                        if ic % 2 == 1:
                            nc.gpsimd.collective_compute(
                                "AllReduce", ADD, replica_groups=rg,
                                ins=[ar_in[l][g][hh]],
                                outs=[ar_out[l][g][hh]])

                def xsage(g):
                    """X-only SAGE terms: M @ X @ Wls + X_own @ Wrs (+ bl).
                    Independent of the attention output; fills PE slack."""
                    ybig = y_pool.tile([128, NJ * 128], BF,
                                       name=f"y_{l}_{g}", tag=f"y{g}")
                    for jb in range(NJ // 4):
                        psy = pscs_pool.tile([128, 512], F32, tag="ps_cs",
                                             name=f"psy_{l}_{g}_{jb}")
                        for k in range(4):
                            jt = jb * 4 + k
                            nc.tensor.matmul(psy[:, k * 128:(k + 1) * 128],
                                             xgt[g][:, jt * 128:(jt + 1) * 128],
                                             W(l, WLS), start=True, stop=True)
                        nc.vector.tensor_copy(
                            ybig[:, jb * 512:(jb + 1) * 512], psy[:])
                    ps_a = psp_pool.tile([128, 512], F32, tag="ps_p",
                                         name=f"psa_{l}_{g}")
                    for jb in range(NJ // 4):
                        mtc_t = mt_pool.tile([128, 4 * SH], BF, tag="mtcs",
                                             bufs=4,
                                             name=f"mtc_{l}_{g}_{jb}")
                        nc.scalar.dma_start(
                            mtc_t.rearrange("p (j n) -> p j n", j=4),
                            mtc_in[g].ap()[4 * jb:4 * jb + 4]
                            .rearrange("j p n -> p j n"))
                        for k in range(4):
                            jt = jb * 4 + k
                            nc.tensor.matmul(
                                ps_a[:], ybig[:, jt * 128:(jt + 1) * 128],
                                mtc_t[:, k * SH:(k + 1) * SH],
                                start=(jt == 0), stop=False)
                    nc.tensor.matmul(ps_a[:], W(l, WRS), xown[g][:],
                                     start=False, stop=True)
                    t = spool.tile([128, SH], F32, name=f"ls_{l}_{g}",
                                   tag=f"ls{g}")
                    nc.vector.tensor_scalar(t[:], ps_a[:], B(l, BL), None,
                                            ADD)
                    ls[g] = t

                def tail(g):
                    """Post-AllReduce: softmax epilogue on the full width,
                    local out-part aggregation, h for own rows, AllGather."""
                    pfull = spool.tile([128, N], BF, name=f"pfull_{l}_{g}",
                                       tag="pfull", bufs=1)
                    csrow = spool.tile([1, N], BF, name=f"csrow_{l}_{g}",
                                       tag="csrow")
                    outt = spool.tile([128, N], BF, name=f"outt_{l}_{g}",
                                      tag="outt", bufs=1)
                    for hh in range(2):
                        nc.scalar.dma_start(
                            pfull[:, hh * 2048:(hh + 1) * 2048],
                            ar_out[l][g][hh, 0:128, :])
                        nc.scalar.dma_start(
                            csrow[:, hh * 2048:(hh + 1) * 2048],
                            ar_out[l][g][hh, 128:129, :])
                        for chq in range(4):
                            ch = hh * 4 + chq
                            sl = slice(ch * 512, (ch + 1) * 512)
                            ps_rep = psp_pool.tile([128, 512], F32,
                                                   tag="ps_p",
                                                   name=f"psrep_{l}_{g}_{ch}")
                            nc.tensor.matmul(ps_rep[:], ones_row[:],
                                             csrow[:, sl],
                                             start=True, stop=True)
                            rr = spool.tile([128, 512], F32,
                                            name=f"rr_{l}_{g}", tag="rr")
                            nc.vector.reciprocal_approx_fast(rr[:], ps_rep[:])
                            prod = spool.tile([128, 512], BF,
                                              name=f"prod_{l}_{g}",
                                              tag="prod")
                            nc.vector.tensor_tensor(prod[:], pfull[:, sl],
                                                    rr[:], MULT)
                            nc.vector.tensor_scalar(outt[:, sl], prod[:],
                                                    B(l, BV), None, ADD)
                    # own columns of out (runtime core offset)
                    oown = spool.tile([128, 512], BF, name=f"oown_{l}_{g}",
                                      tag="oown")
                    pid = nc.gpsimd.partition_id()
                    nc.gpsimd.dma_start(oown[:],
                                        outt[:, bass.ds(pid * SH, SH)])
                    # U = out @ Wl1  (natural [j, d] tiles, all 4096 j)
                    ubig = spool.tile([128, N], BF, name=f"ubig_{l}_{g}",
                                      tag="ubig", bufs=1)
                    for jb in range(8):
                        ps_u = pscs_pool.tile([128, 512], F32, tag="ps_cs",
                                              name=f"psu_{l}_{g}_{jb}")
                        for k in range(4):
                            jt = jb * 4 + k
                            nc.tensor.matmul(
                                ps_u[:, k * 128:(k + 1) * 128],
                                outt[:, jt * 128:(jt + 1) * 128],
                                W(l, WL1), start=True, stop=True)
                        nc.vector.tensor_copy(
                            ubig[:, jb * 512:(jb + 1) * 512], ps_u[:])
                    # P2 = M @ U |own + out_own @ Wr1   (to subtract from ls)
                    ps_a2 = psp_pool.tile([128, 512], F32, tag="ps_p",
                                          name=f"psa2_{l}_{g}")
                    for jb in range(NJ // 4):
                        mtc_t = mt_pool.tile([128, 4 * SH], BF, tag="mtcs",
                                             bufs=4,
                                             name=f"mtc2_{l}_{g}_{jb}")
                        nc.scalar.dma_start(
                            mtc_t.rearrange("p (j n) -> p j n", j=4),
                            mtc_in[g].ap()[4 * jb:4 * jb + 4]
                            .rearrange("j p n -> p j n"))
                        for k in range(4):
                            jt = jb * 4 + k
                            nc.tensor.matmul(
                                ps_a2[:], ubig[:, jt * 128:(jt + 1) * 128],
                                mtc_t[:, k * SH:(k + 1) * SH],
                                start=(jt == 0), stop=False)
                    nc.tensor.matmul(ps_a2[:], W(l, WR1N), oown[:],
                                     start=False, stop=True)
                    h = spool.tile([D, SH], BF, name=f"hown_{l}_{g}",
                                   tag=f"xo{g}")
                    if l == 0:
                        t2 = spool.tile([128, 512], F32, name=f"t2_{l}_{g}",
                                        tag="t2")
                        nc.vector.tensor_tensor(t2[:], ls[g][:], ps_a2[:],
                                                SUB)
                        nc.vector.tensor_scalar(h[:], t2[:], 0.0, None, MAX)
                    else:
                        nc.vector.tensor_tensor(h[:], ls[g][:], ps_a2[:],
                                                SUB)
                    hown[g] = h
                    nc.scalar.dma_start(hag_in[l][g][:], h[:])
                    nc.gpsimd.collective_compute(
                        "AllGather", mybir.AluOpType.bypass, replica_groups=rg,
                        ins=[hag_in[l][g][:]], outs=[hag_out[l][g][:]])

                xsage(0)
                xsage(1)
                attention(0)
                tail(0)
                attention(1)
                tail(1)

                # ---- gather new X generation ----
                new_xgt = []
                for g in range(2):
                    t = xt_pool.tile([D, N], BF, name=f"x{g}t_{l + 1}",
                                     tag=f"xt{g}")
                    nc.scalar.dma_start(
                        t.rearrange("p (c n) -> p c n", c=NCORES),
                        hag_out[l][g].ap().rearrange("c p n -> p c n"))
                    new_xgt.append(t)
                xgt = new_xgt
                xown = hown
                if l == 1:
                    hown_final = hown

            # ---- final adjacency: sigmoid(F @ F^T), own 1024 rows ----
            for g in range(2):
                for rt in range(4):
                    lhs = hown_final[g][:, rt * 128:(rt + 1) * 128]
                    for cb in range(8):
                        src = xgt[cb // 4]
                        c0 = (cb % 4) * ICW
                        ps_z = ps_pool.tile([128, ICW], F32, tag="ps")
                        for h in range(2):
                            nc.tensor.matmul(
                                ps_z[:, h * 512:(h + 1) * 512], lhs,
                                src[:, c0 + h * 512:c0 + (h + 1) * 512],
                                start=True, stop=True)
                        z = z_pool.tile([128, ICW], F32, tag="z")
                        nc.scalar.activation(
                            z[:], ps_z[:],
                            mybir.ActivationFunctionType.Sigmoid)
                        nc.scalar.dma_start(
                            out_ext[g, rt * 128:(rt + 1) * 128,
                                    cb * ICW:cb * ICW + 512],
                            z[:, 0:512])
                        nc.gpsimd.dma_start(
                            out_ext[g, rt * 128:(rt + 1) * 128,
                                    cb * ICW + 512:(cb + 1) * ICW],
                            z[:, 512:1024])

    nc.compile()
    return nc


def _host_prep(inputs):
    """Build per-core input maps from the full problem inputs."""
    x1 = np.asarray(inputs["x1"], np.float32)
    x2 = np.asarray(inputs["x2"], np.float32)
    x1t = np.ascontiguousarray(x1.T).astype(BF16)
    x2t = np.ascontiguousarray(x2.T).astype(BF16)

    def norm_adj_t(ei):
        ei = np.asarray(ei)
        A = np.zeros((N, N), np.float32)
        np.add.at(A, (ei[1], ei[0]), 1.0)
        deg = A.sum(1)
        A /= np.maximum(deg, 1.0)[:, None]
        return np.ascontiguousarray(A.T)  # MT[j, n]

    mt = [norm_adj_t(inputs["ei1"]), norm_adj_t(inputs["ei2"])]

    wm = np.zeros((15, 128, 128), np.float32)
    bs = np.zeros((8, 128, 1), np.float32)
    for l, s in enumerate(("1", "2")):
        wm[7 * l + WK] = inputs["Wk" + s]
        wm[7 * l + WQ] = inputs["Wq" + s]
        wm[7 * l + WV] = inputs["Wv" + s]
        wm[7 * l + WLS] = inputs["Wl" + s][:128] + inputs["Wl" + s][128:]
        wm[7 * l + WL1] = inputs["Wl" + s][128:]
        wm[7 * l + WRS] = inputs["Wr" + s][:128] + inputs["Wr" + s][128:]
        wm[7 * l + WR1N] = inputs["Wr" + s][128:]
        bs[4 * l + BK, :, 0] = inputs["bk" + s]
        bs[4 * l + BQ, :, 0] = inputs["bq" + s]
        bs[4 * l + BV, :, 0] = inputs["bv" + s]
        bs[4 * l + BL, :, 0] = inputs["bl" + s]
    wm[IDENT] = np.eye(128)
    wm = wm.astype(BF16)

    in_maps = []
    for c in range(NCORES):
        sl = slice(c * SH, (c + 1) * SH)
        in_maps.append({
            "x1t": x1t,
            "x2t": x2t,
            "xown": np.stack([x1t[:, sl], x2t[:, sl]]),
            "mtc1": np.ascontiguousarray(
                mt[0][:, sl].astype(BF16).reshape(NJ, 128, SH)),
            "mtc2": np.ascontiguousarray(
                mt[1][:, sl].astype(BF16).reshape(NJ, 128, SH)),
            "wm": wm,
            "bs": bs,
        })
    return in_maps


def _assemble(results):
    full = np.empty((2 * N, 2 * N), np.float32)
    for c in range(NCORES):
        o = results[c]["out"]
        full[c * SH:(c + 1) * SH] = o[0]
        full[N + c * SH:N + (c + 1) * SH] = o[1]
    return full


def get_nc():
    if "nc" not in _cache:
        _cache["nc"] = _build_nc()
    return _cache["nc"]


def kernel(**inputs):
    nc = get_nc()
    in_maps = _host_prep(inputs)
    res = run_bass_kernel_spmd(nc, in_maps, core_ids=list(range(NCORES)))
    return _assemble(res.results)



# revision 3
# speedup vs baseline: 1.1870x; 1.1870x over previous
"""Trainium2 Bass kernel for nn_AttMatch (2-graph attention + SAGEConv GNN).

Self-contained: takes the full unsharded inputs of the reference problem,
shards across 8 NeuronCores internally, runs one SPMD NEFF, and gathers the
full [8192, 8192] sigmoid adjacency output.

v2 vs the original baseline (630 us):
  * layer phases reordered (proj -> attn(0) -> attn(1) -> xsage(0) ->
    tail(0) -> xsage(1) -> tail(1)) so the AllReduces hide under the other
    graph's attention + the X-only SAGE work, and each AllGather hides under
    the other graph's tail / next-layer projections.
  * exp(scores) is split between the scalar engine (exact Exp -> fp8 out)
    and the DVE (Schraudolph bit-trick exp -> uint8 viewed as fp8e4m3).
  * alpha^T@v and the softmax column sums run as fp8 DoubleRow matmuls
    (K=256 per instruction); the column sum uses a fp8 ones stationary and
    accumulates in PSUM, removing all vector-engine partial-sum adds.
  * the mean-aggregation operator M^T is resident in SBUF as fp8 (loaded
    once), and both M-aggregations per layer run as fp8 DoubleRow matmuls
    against fp8 Y/U tiles.
  * the final sigmoid(F@F^T) exploits symmetry: each core computes a
    balanced circulant 9/16 of the block matrix (its own 2x512 rows x 9
    column units); the host mirrors the rest. Sigmoid splits between the
    scalar engine and a DVE Schraudolph sigmoid chain; output is bf16,
    upcast on the host.
"""

import numpy as np
import ml_dtypes

import concourse.bass as bass
import concourse.bacc as bacc
import concourse.tile as tile
import concourse.mybir as mybir
from concourse.bass_utils import run_bass_kernel_spmd

BF16 = ml_dtypes.bfloat16
E4M3 = ml_dtypes.float8_e4m3

N = 4096          # nodes per graph
D = 128           # feature dim
NCORES = 8
SH = N // NCORES  # 512 nodes per graph per core
ICW = 1024        # query-chunk width
NIC = N // ICW    # 4 query chunks
NT = 2 * SH // 128   # 8 local target tiles (512 of each graph)
NTP = NT // 2     # 4 target-tile pairs (DoubleRow)
NJ = N // 128     # 32 source-node tiles
NJP = NJ // 2     # 16 source-tile pairs
KU = 9            # circulant column units in the final phase
INV_SCALE = 1.0 / np.sqrt(128.0)

LOG2E = 1.4426950408889634
# DVE Schraudolph exp -> fp8e4m3 bits (bias 7, 3 mantissa bits):
#   bits = round(scores * INV_SCALE * 8*log2(e) + 8*7 - corr)
ES_A = INV_SCALE * 8.0 * LOG2E
ES_B = 56.0 - 0.344 + 0.5
# DVE Schraudolph sigmoid: e^{-x} via f32 bit trick, then 1/(1+u)
SIG_A = (1 << 23) * LOG2E
SIG_B = 127.0 * (1 << 23) - 360768.0
SIG_LO = -1.0e9          # clamp of -x*SIG_A (x >> 0 -> e^-x = tiny)
SIG_HI = 2.09e9          # clamp of bits before int32 convert (x << 0)

F32 = mybir.dt.float32
BF = mybir.dt.bfloat16
FP8 = mybir.dt.float8e4
U8 = mybir.dt.uint8
DR = mybir.MatmulPerfMode.DoubleRow

ADD = mybir.AluOpType.add
SUB = mybir.AluOpType.subtract
MULT = mybir.AluOpType.mult
MAX = mybir.AluOpType.max
MIN = mybir.AluOpType.min
EXP = mybir.ActivationFunctionType.Exp
IDN = mybir.ActivationFunctionType.Identity
SIG = mybir.ActivationFunctionType.Sigmoid

# wm indices (per layer l: base = 7*l)
WK, WQ, WV, WLS, WL1, WRS, WR1N = range(7)
IDENT = 14
# bias indices (per layer l: base = 4*l)
BK, BQ, BV, BL = range(4)

_cache = {}


def _build_nc():
    nc = bacc.Bacc("TRN2", target_bir_lowering=False, debug=False,
                   num_devices=NCORES)

    # ---- external I/O ----
    x1t = nc.dram_tensor("x1t", [D, N], BF, kind="ExternalInput")
    x2t = nc.dram_tensor("x2t", [D, N], BF, kind="ExternalInput")
    xgt_in = [x1t, x2t]
    xown_in = nc.dram_tensor("xown", [2, D, SH], BF, kind="ExternalInput")
    mtc_in = [nc.dram_tensor("mtc1", [NJ, 128, SH], FP8, kind="ExternalInput"),
              nc.dram_tensor("mtc2", [NJ, 128, SH], FP8, kind="ExternalInput")]
    wm_in = nc.dram_tensor("wm", [15, 128, 128], BF, kind="ExternalInput")
    bs_in = nc.dram_tensor("bs", [8, 128, 1], F32, kind="ExternalInput")
    out_ext = nc.dram_tensor("out", [2, SH, KU * 512], BF,
                             kind="ExternalOutput")

    # ---- internal DRAM for collectives ----
    rg = [list(range(NCORES))]
    ar_in = [[nc.dram_tensor(f"ar_in_{l}_{g}", [2, 129, N // 2], BF)
              for g in range(2)] for l in range(2)]
    ar_out = [[nc.dram_tensor(f"ar_out_{l}_{g}", [2, 129, N // 2], BF,
                              addr_space="Shared")
               for g in range(2)] for l in range(2)]
    hag_in = [[nc.dram_tensor(f"hag_in_{l}_{g}", [D, SH], BF)
               for g in range(2)] for l in range(2)]
    hag_out = [[nc.dram_tensor(f"hag_out_{l}_{g}", [NCORES, D, SH], BF,
                               addr_space="Shared")
                for g in range(2)] for l in range(2)]

    with tile.TileContext(nc) as tc:
        with (
            tc.tile_pool(name="const", bufs=1) as cpool,
            tc.tile_pool(name="mt", bufs=1) as mt_pool,
            tc.tile_pool(name="xt", bufs=2) as xt_pool,
            tc.tile_pool(name="xsel", bufs=1) as xsel_pool,
            tc.tile_pool(name="kq", bufs=2) as kq_pool,
            tc.tile_pool(name="es", bufs=4) as es_pool,
            tc.tile_pool(name="st", bufs=2) as st_pool,
            tc.tile_pool(name="tl", bufs=1) as tl_pool,
            tc.tile_pool(name="yu", bufs=2) as yu_pool,
            tc.tile_pool(name="sm", bufs=2) as sm_pool,
            tc.tile_pool(name="fz", bufs=4) as fz_pool,
            tc.tile_pool(name="psS", bufs=2, space="PSUM") as psS,
            tc.tile_pool(name="psP", bufs=3, space="PSUM") as psP,
            tc.tile_pool(name="psC", bufs=2, space="PSUM") as psC,
            tc.tile_pool(name="psT", bufs=1, space="PSUM") as psT,
        ):
            pid = nc.gpsimd.partition_id()

            # ---- constants ----
            wm = cpool.tile([128, 15 * 128], BF, name="wm_sb")
            nc.scalar.dma_start(
                wm.rearrange("p (i f) -> p i f", i=15),
                wm_in.ap().rearrange("i p f -> p i f"))
            bs = cpool.tile([128, 8], F32, name="bs_sb")
            nc.scalar.dma_start(
                bs.rearrange("p (i f) -> p i f", i=8),
                bs_in.ap().rearrange("i p f -> p i f"))
            ones_row = cpool.tile([1, 128], BF, name="ones_row")
            nc.vector.memset(ones_row[:], 1.0)
            ones8 = cpool.tile([128, 256], FP8, name="ones8")
            nc.vector.memset(ones8[:], 1.0)

            def W(l, i):
                base = 7 * l + i if i < 7 else IDENT
                return wm[:, 128 * base:128 * (base + 1)]

            def B(l, i):
                return bs[:, 4 * l + i:4 * l + i + 1]

            ident = wm[:, 128 * IDENT:128 * (IDENT + 1)]

            # ---- initial loads ----
            xgt = []
            for g in range(2):
                t = xt_pool.tile([D, N], BF, name=f"x{g}t_0", tag=f"xt{g}")
                nc.scalar.dma_start(t[:], xgt_in[g][:])
                xgt.append(t)
            xown = []
            for g in range(2):
                t = sm_pool.tile([D, SH], BF, name=f"xown{g}_0", tag=f"xo{g}")
                nc.scalar.dma_start(t[:], xown_in[g])
                xown.append(t)
            # resident M^T (fp8), loaded once per graph
            mt = []
            for g in range(2):
                t = mt_pool.tile([128, NJ * SH], FP8, name=f"mt{g}")
                nc.scalar.dma_start(
                    t.rearrange("p (j n) -> p j n", j=NJ),
                    mtc_in[g].ap().rearrange("j p n -> p j n"))
                mt.append(t)
            # final-phase gathered features (both graphs + wraparound)
            xsel = xsel_pool.tile([128, 12288], BF, name="xsel")

            hown_final = [None, None]
            state = {"xgt": xgt, "xown": xown}

            def proj(l):
                """k/v/q projections. k,v need only local rows (no AG dep);
                q needs the full X of this generation."""
                xgt, xown = state["xgt"], state["xown"]
                kt = kq_pool.tile([D, 2 * SH], BF, name=f"kt_{l}", tag="kt")
                vnat = kq_pool.tile([128, NT * 128], FP8, name=f"vn_{l}",
                                    tag="vn")
                for g in range(2):
                    ps = psS.tile([128, 512], F32, tag="psS",
                                  name=f"psk_{l}_{g}")
                    nc.tensor.matmul(ps[:], W(l, WK), xown[g][:],
                                     start=True, stop=True)
                    nc.vector.tensor_scalar(kt[:, g * SH:(g + 1) * SH], ps[:],
                                            B(l, BK), None, ADD)
                    ps2 = psS.tile([128, 512], F32, tag="psS",
                                   name=f"psv_{l}_{g}")
                    nc.tensor.matmul(ps2[:], W(l, WV), xown[g][:],
                                     start=True, stop=True)
                    vt = st_pool.tile([128, SH], BF, name=f"vt_{l}_{g}",
                                      tag="vt")
                    nc.vector.tensor_copy(vt[:], ps2[:])
                    pst = psT.tile([128, 512], BF, tag="psT",
                                   name=f"pst_{l}_{g}")
                    for j in range(4):
                        nc.tensor.transpose(pst[:, j * 128:(j + 1) * 128],
                                            vt[:, j * 128:(j + 1) * 128],
                                            ident)
                    nc.vector.tensor_copy(vnat[:, g * 512:(g + 1) * 512],
                                          pst[:])
                qt = []
                for g in range(2):
                    q = kq_pool.tile([D, N], BF, name=f"qt_{l}_{g}",
                                     tag=f"qt{g}", bufs=1)
                    for c in range(8):
                        psq = psS.tile([128, 512], F32, tag="psS",
                                       name=f"psq_{l}_{g}_{c}")
                        nc.tensor.matmul(psq[:], W(l, WQ),
                                         xgt[g][:, c * 512:(c + 1) * 512],
                                         start=True, stop=True)
                        if g == 0:
                            nc.vector.tensor_scalar(
                                q[:, c * 512:(c + 1) * 512], psq[:],
                                B(l, BQ), None, ADD)
                        else:
                            nc.scalar.activation(
                                q[:, c * 512:(c + 1) * 512], psq[:], IDN,
                                bias=B(l, BQ))
                    qt.append(q)
                return kt, vnat, qt

            def attention(l, g, kt, vnat, qt):
                """Per query-chunk: scores -> exp (fp8) -> DR pv + DR colsum;
                AllReduce per 2048-query half."""
                for ic in range(NIC):
                    dve = ic % 2 == 0
                    php = [psP.tile([128, 512], F32, tag="psP",
                                    name=f"php{h}_{l}_{g}_{ic}")
                           for h in range(2)]
                    pcs = [psC.tile([128, 512], F32, tag="psC",
                                    name=f"pcs{h}_{l}_{g}_{ic}")
                           for h in range(2)]
                    for tt2 in range(NTP):
                        es = [es_pool.tile([128, 1024], FP8, tag="es",
                                           name=f"es{h}_{l}_{g}_{ic}_{tt2}")
                              for h in range(2)]
                        for j in range(2):
                            tt = 2 * tt2 + j
                            for h in range(2):
                                ps_s = psS.tile([128, 512], F32, tag="psS",
                                                name=f"pss_{l}_{g}_{ic}_{tt}_{h}")
                                nc.tensor.matmul(
                                    ps_s[:], kt[:, tt * 128:(tt + 1) * 128],
                                    qt[g][:, ic * ICW + h * 512:
                                            ic * ICW + (h + 1) * 512],
                                    start=True, stop=True)
                                dst = es[h][:, j * 512:(j + 1) * 512]
                                if dve:
                                    nc.vector.tensor_scalar(
                                        dst.bitcast(U8), ps_s[:],
                                        ES_A, ES_B, MULT, ADD)
                                else:
                                    nc.scalar.activation(dst, ps_s[:], EXP,
                                                         scale=INV_SCALE)
                        for h in range(2):
                            esh = es[h].rearrange("p (k n) -> p k n", k=2)
                            nc.tensor.matmul(
                                php[h][:],
                                vnat[:, tt2 * 256:(tt2 + 1) * 256]
                                .rearrange("p (k m) -> p k m", k=2),
                                esh, start=(tt2 == 0), stop=(tt2 == NTP - 1),
                                perf_mode=DR)
                            nc.tensor.matmul(
                                pcs[h][:],
                                ones8.rearrange("p (k m) -> p k m", k=2),
                                esh, start=(tt2 == 0), stop=(tt2 == NTP - 1),
                                perf_mode=DR)
                    pc = st_pool.tile([128, ICW], BF, tag="pc")
                    cc = st_pool.tile([1, ICW], BF, tag="cc")
                    for h in range(2):
                        nc.vector.tensor_copy(pc[:, h * 512:(h + 1) * 512],
                                              php[h][:])
                        nc.vector.tensor_copy(cc[:, h * 512:(h + 1) * 512],
                                              pcs[h][0:1, :])
                    hh, icq = divmod(ic, 2)
                    nc.scalar.dma_start(
                        ar_in[l][g][hh, 0:128, icq * ICW:(icq + 1) * ICW],
                        pc[:])
                    nc.scalar.dma_start(
                        ar_in[l][g][hh, 128:129, icq * ICW:(icq + 1) * ICW],
                        cc[:])
                    if icq == 1:
                        nc.gpsimd.collective_compute(
                            "AllReduce", ADD, replica_groups=rg,
                            ins=[ar_in[l][g][hh]], outs=[ar_out[l][g][hh]])

            ls = [None, None]

            def xsage(l, g):
                """X-only SAGE terms: M @ (X@Wls) + X_own @ Wrs + bl."""
                xgt, xown = state["xgt"], state["xown"]
                yb = yu_pool.tile([128, N], FP8, name=f"yb_{l}_{g}", tag="yb")
                for jb in range(8):
                    psy = psS.tile([128, 512], F32, tag="psS",
                                   name=f"psy_{l}_{g}_{jb}")
                    for k in range(4):
                        jt = jb * 4 + k
                        nc.tensor.matmul(psy[:, k * 128:(k + 1) * 128],
                                         xgt[g][:, jt * 128:(jt + 1) * 128],
                                         W(l, WLS), start=True, stop=True)
                    nc.vector.tensor_copy(yb[:, jb * 512:(jb + 1) * 512],
                                          psy[:])
                ps_a = psP.tile([128, 512], F32, tag="psP",
                                name=f"psa_{l}_{g}")
                for jp in range(NJP):
                    nc.tensor.matmul(
                        ps_a[:],
                        yb[:, jp * 256:(jp + 1) * 256]
                        .rearrange("p (k m) -> p k m", k=2),
                        mt[g][:, jp * 1024:(jp + 1) * 1024]
                        .rearrange("p (k n) -> p k n", k=2),
                        start=(jp == 0), stop=False, perf_mode=DR,
                        skip_group_check=True)
                nc.tensor.matmul(ps_a[:], W(l, WRS), xown[g][:],
                                 start=False, stop=True,
                                 skip_group_check=True)
                t = sm_pool.tile([128, SH], F32, name=f"ls_{l}_{g}",
                                 tag=f"ls{g}", bufs=1)
                nc.vector.tensor_scalar(t[:], ps_a[:], B(l, BL), None, ADD)
                ls[g] = t

            def tail(l, g):
                """Post-AllReduce softmax epilogue, out-dependent SAGE part,
                h for own rows, AllGather + next-gen feature loads."""
                pfull = tl_pool.tile([128, N], BF, name=f"pfull_{l}_{g}",
                                     tag="pfull")
                csrow = tl_pool.tile([1, N], BF, name=f"csrow_{l}_{g}",
                                     tag="csrow")
                outt = tl_pool.tile([128, N], BF, name=f"outt_{l}_{g}",
                                    tag="outt")
                for hh in range(2):
                    nc.scalar.dma_start(pfull[:, hh * 2048:(hh + 1) * 2048],
                                        ar_out[l][g][hh, 0:128, :])
                    nc.scalar.dma_start(csrow[:, hh * 2048:(hh + 1) * 2048],
                                        ar_out[l][g][hh, 128:129, :])
                for ch in range(8):
                    sl = slice(ch * 512, (ch + 1) * 512)
                    ps_rep = psS.tile([128, 512], F32, tag="psS",
                                      name=f"psrep_{l}_{g}_{ch}")
                    nc.tensor.matmul(ps_rep[:], ones_row[:], csrow[:, sl],
                                     start=True, stop=True)
                    rr = st_pool.tile([128, 512], F32, name=f"rr_{l}_{g}",
                                      tag="rr")
                    nc.vector.reciprocal_approx_fast(rr[:], ps_rep[:])
                    prod = st_pool.tile([128, 512], BF, name=f"prod_{l}_{g}",
                                        tag="prod")
                    nc.vector.tensor_tensor(prod[:], pfull[:, sl], rr[:],
                                            MULT)
                    nc.scalar.activation(outt[:, sl], prod[:], IDN,
                                         bias=B(l, BV))
                oown = st_pool.tile([128, 512], BF, name=f"oown_{l}_{g}",
                                    tag="oown")
                nc.gpsimd.dma_start(oown[:], outt[:, bass.ds(pid * SH, SH)])
                ub = yu_pool.tile([128, N], FP8, name=f"ub_{l}_{g}", tag="ub")
                for jb in range(8):
                    psu = psS.tile([128, 512], F32, tag="psS",
                                   name=f"psu_{l}_{g}_{jb}")
                    for k in range(4):
                        jt = jb * 4 + k
                        nc.tensor.matmul(psu[:, k * 128:(k + 1) * 128],
                                         outt[:, jt * 128:(jt + 1) * 128],
                                         W(l, WL1), start=True, stop=True)
                    nc.vector.tensor_copy(ub[:, jb * 512:(jb + 1) * 512],
                                          psu[:])
                ps_a2 = psP.tile([128, 512], F32, tag="psP",
                                 name=f"psa2_{l}_{g}")
                for jp in range(NJP):
                    nc.tensor.matmul(
                        ps_a2[:],
                        ub[:, jp * 256:(jp + 1) * 256]
                        .rearrange("p (k m) -> p k m", k=2),
                        mt[g][:, jp * 1024:(jp + 1) * 1024]
                        .rearrange("p (k n) -> p k n", k=2),
                        start=(jp == 0), stop=False, perf_mode=DR,
                        skip_group_check=True)
                nc.tensor.matmul(ps_a2[:], W(l, WR1N), oown[:],
                                 start=False, stop=True,
                                 skip_group_check=True)
                h = sm_pool.tile([D, SH], BF, name=f"hown_{l}_{g}",
                                 tag=f"xo{g}")
                if l == 0:
                    t2 = st_pool.tile([128, 512], F32, name=f"t2_{l}_{g}",
                                      tag="rr")
                    nc.vector.tensor_tensor(t2[:], ls[g][:], ps_a2[:], SUB)
                    nc.vector.tensor_scalar(h[:], t2[:], 0.0, None, MAX)
                else:
                    nc.vector.tensor_tensor(h[:], ls[g][:], ps_a2[:], SUB)
                nc.scalar.dma_start(hag_in[l][g][:], h[:])
                nc.gpsimd.collective_compute(
                    "AllGather", mybir.AluOpType.bypass, replica_groups=rg,
                    ins=[hag_in[l][g][:]], outs=[hag_out[l][g][:]])
                # earliest possible load of the gathered next-gen features
                if l == 0:
                    t = xt_pool.tile([D, N], BF, name=f"x{g}t_1",
                                     tag=f"xt{g}")
                    nc.gpsimd.dma_start(
                        t.rearrange("p (c n) -> p c n", c=NCORES),
                        hag_out[l][g].ap().rearrange("c p n -> p c n"))
                    state["xgt"][g] = t
                else:
                    nc.gpsimd.dma_start(
                        xsel[:, g * N:(g + 1) * N]
                        .rearrange("p (c n) -> p c n", c=NCORES),
                        hag_out[l][g].ap().rearrange("c p n -> p c n"))
                    if g == 0:
                        nc.gpsimd.dma_start(
                            xsel[:, 2 * N:3 * N]
                            .rearrange("p (c n) -> p c n", c=NCORES),
                            hag_out[l][g].ap().rearrange("c p n -> p c n"))
                return h

            # ================= layers =================
            for l in range(2):
                kt, vnat, qt = proj(l)
                attention(l, 0, kt, vnat, qt)
                attention(l, 1, kt, vnat, qt)
                xsage(l, 0)
                h0 = tail(l, 0)
                xsage(l, 1)
                h1 = tail(l, 1)
                state["xown"] = [h0, h1]
                if l == 1:
                    hown_final = [h0, h1]

            # ================= final adjacency (circulant symmetric) ========
            def sig_block(ps_z, z):
                """z[bf16] = sigmoid(ps_z) via DVE Schraudolph chain."""
                sg1 = fz_pool.tile([128, 512], F32, tag="sg")
                nc.vector.tensor_scalar(sg1[:], ps_z[:], -SIG_A, SIG_LO,
                                        MULT, MAX)
                sg2 = fz_pool.tile([128, 512], F32, tag="sg")
                nc.vector.tensor_scalar(sg2[:].bitcast(mybir.dt.int32),
                                        sg1[:], SIG_B, SIG_HI, ADD, MIN)
                sg3 = fz_pool.tile([128, 512], F32, tag="sg")
                nc.vector.tensor_scalar(sg3[:], sg2[:], 1.0, None, ADD)
                sg4 = fz_pool.tile([128, 512], F32, tag="sg")
                nc.vector.reciprocal_approx_fast(sg4[:], sg3[:])
                nc.vector.tensor_copy(z[:], sg4[:])

            def fin_block(g, k, rhs):
                for rt in range(4):
                    ps_z = psS.tile([128, 512], F32, tag="psS",
                                    name=f"psz_{g}_{k}_{rt}")
                    nc.tensor.matmul(
                        ps_z[:],
                        hown_final[g][:, rt * 128:(rt + 1) * 128],
                        rhs, start=True, stop=True)
                    z = fz_pool.tile([128, 512], BF, tag="z")
                    if rt == 2:
                        sig_block(ps_z, z)
                    else:
                        nc.scalar.activation(z[:], ps_z[:], SIG)
                    nc.scalar.dma_start(
                        out_ext[g, rt * 128:(rt + 1) * 128,
                                k * 512:(k + 1) * 512],
                        z[:])

            # k=0 (own-column diagonal blocks) first: no AllGather dependency
            for g in range(2):
                fin_block(g, 0, hown_final[g][:])
            for k in range(1, KU):
                for g in range(2):
                    stg = st_pool.tile([128, 512], BF, tag="fstage", bufs=3,
                                       name=f"stg_{g}_{k}")
                    nc.gpsimd.dma_start(
                        stg[:],
                        xsel[:, bass.ds((pid + g * 8 + k) * 512, 512)])
                    fin_block(g, k, stg[:])

    nc.compile()
    return nc


def _host_prep(inputs):
    """Build per-core input maps from the full problem inputs."""
    x1 = np.asarray(inputs["x1"], np.float32)
    x2 = np.asarray(inputs["x2"], np.float32)
    x1t = np.ascontiguousarray(x1.T).astype(BF16)
    x2t = np.ascontiguousarray(x2.T).astype(BF16)

    def norm_adj_t(ei):
        ei = np.asarray(ei)
        A = np.zeros((N, N), np.float32)
        np.add.at(A, (ei[1], ei[0]), 1.0)
        deg = A.sum(1)
        A /= np.maximum(deg, 1.0)[:, None]
        return np.ascontiguousarray(A.T)  # MT[j, n]

    mt = [norm_adj_t(inputs["ei1"]), norm_adj_t(inputs["ei2"])]

    wm = np.zeros((15, 128, 128), np.float32)
    bs = np.zeros((8, 128, 1), np.float32)
    for l, s in enumerate(("1", "2")):
        wm[7 * l + WK] = inputs["Wk" + s]
        wm[7 * l + WQ] = inputs["Wq" + s]
        wm[7 * l + WV] = inputs["Wv" + s]
        wm[7 * l + WLS] = inputs["Wl" + s][:128] + inputs["Wl" + s][128:]
        wm[7 * l + WL1] = inputs["Wl" + s][128:]
        wm[7 * l + WRS] = inputs["Wr" + s][:128] + inputs["Wr" + s][128:]
        wm[7 * l + WR1N] = inputs["Wr" + s][128:]
        bs[4 * l + BK, :, 0] = inputs["bk" + s]
        bs[4 * l + BQ, :, 0] = inputs["bq" + s]
        bs[4 * l + BV, :, 0] = inputs["bv" + s]
        bs[4 * l + BL, :, 0] = inputs["bl" + s]
    wm[IDENT] = np.eye(128)
    wm = wm.astype(BF16)

    in_maps = []
    for c in range(NCORES):
        sl = slice(c * SH, (c + 1) * SH)
        in_maps.append({
            "x1t": x1t,
            "x2t": x2t,
            "xown": np.stack([x1t[:, sl], x2t[:, sl]]),
            "mtc1": np.ascontiguousarray(
                mt[0][:, sl]).astype(E4M3).reshape(NJ, 128, SH),
            "mtc2": np.ascontiguousarray(
                mt[1][:, sl]).astype(E4M3).reshape(NJ, 128, SH),
            "wm": wm,
            "bs": bs,
        })
    return in_maps


def _assemble(results):
    """Place each core's circulant 9/16 blocks, mirror the rest."""
    full = np.empty((2 * N, 2 * N), np.float32)
    filled = np.zeros((16, 16), bool)
    for c in range(NCORES):
        o = np.asarray(results[c]["out"]).astype(np.float32)
        for gi in range(2):
            u = c + 8 * gi
            rows = slice(u * 512, (u + 1) * 512)
            for k in range(KU):
                cu = (u + k) % 16
                full[rows, cu * 512:(cu + 1) * 512] = \
                    o[gi][:, k * 512:(k + 1) * 512]
                filled[u, cu] = True
    for a in range(16):
        for b in range(16):
            if not filled[a, b]:
                full[a * 512:(a + 1) * 512, b * 512:(b + 1) * 512] = \
                    full[b * 512:(b + 1) * 512, a * 512:(a + 1) * 512].T
    return full


def get_nc():
    if "nc" not in _cache:
        _cache["nc"] = _build_nc()
    return _cache["nc"]


def kernel(**inputs):
    nc = get_nc()
    in_maps = _host_prep(inputs)
    res = run_bass_kernel_spmd(nc, in_maps, core_ids=list(range(NCORES)))
    return _assemble(res.results)
